# revision 67
# baseline (speedup 1.0000x reference)
"""Trainium2 Bass kernel for nn_ODEFunc_90159953478502 (MoE routing, inference path).

Math (see reference):
    logits  = x @ Wg[:256] + (t*Wg[512] + bg)      # zeros kill Wg[256:512]
    w       = softmax(logits, axis=-1)             # [B, E]
    eo_e    = tanh(x @ W1[e] + b1[e]) @ W2[e] + b2[e]
    active_e = any_b(w[b,e] > 0.01)
    out     = sum_e active_e * w[:,e,None] * eo_e  # softmax max >= 1/8 > 0.01,
                                                   # so >=1 expert always active

Sharding: expert-parallel. Core e holds the full batch plus only W1[e]/W2[e]
and computes the UNNORMALIZED partial E_e[:,None] * (tanh(x@W1[e]+b1[e]) @
W2[e]) in transposed layout ([D, B]), where E_e = exp(logit_e). Because
out = (sum_e m_e * E_e . eo_e) / S shares one softmax denominator S across
experts, normalization and the 0/1 active mask move to the host-side
unshard: each core exports its exp row (EROW), the host reconstructs
S = sum_e E_e, w = E/S, the mask, and divides once. The b2 rank-1 term
(zero here) is added host-side from a numpy gating pass when nonzero.

Device structure per core (all matmuls bf16, 1 cycle/row on PE; steady-state
matmul cadence measured at 216ns per [128,512]-row stream):
  - x arrives pre-transposed (xT [D, B] bf16) so W1/W2/Wg act as matmul lhsT
    in natural layout (out = lhsT.T @ rhs, contraction on partitions).
  - gating weights are column-permuted per core so the core's own expert is
    partition row 0 of the [E, B] logits: no selector matmuls needed.
  - per 512-chunk: logits^T [8,512] via 2 bf16 matmuls, ACT Exp with fused
    +gbias (no max-subtract: |logits| <= ~4) emitting bf16 into a slice of
    e_all [8, B]; chunk-pair rows are gpsimd partition-broadcast to wb
    [128,1024] bf16 (no DRAM bounce, no S-matmul, no reciprocal).
  - mm2 drains: out^T tile = psum * wb, one DVE multiply per [128,512] tile;
    stores batch 2 chunks per descriptor except the final pair (small
    descriptors keep the post-compute DMA tail short).
  - DMA descriptors drain through shared hardware queues in global issue
    order (~400GB/s aggregate); descriptor count is minimized (3D APs for
    Wg/W2), the first x chunk loads as a small descriptor so the tiny
    constants aren't stuck behind megabyte transfers, and the scalar queue
    issues no DMAs so the in-order ACT chain starts early.
"""

import sys

if "/opt/trn_rl_repo" not in sys.path:
    sys.path.insert(0, "/opt/trn_rl_repo")

import numpy as np

_B, _D, _H, _E = 4096, 256, 1024, 8
_NCORES = 8
_CHUNK = 512
_NCH = _B // _CHUNK
_DT = _D // 128   # 2 d-tiles
_HT = _H // 128   # 8 h-tiles
_THRESH = 0.01

_CACHE = {}


def _build():
    import concourse.bass as bass
    import concourse.tile as tile
    import concourse.mybir as mybir
    from concourse import bacc
    from contextlib import ExitStack

    F32 = mybir.dt.float32
    F32R = mybir.dt.float32r
    BF16 = mybir.dt.bfloat16
    AF = mybir.ActivationFunctionType
    ALU = mybir.AluOpType
    AX = mybir.AxisListType

    nc = bacc.Bacc("TRN2", target_bir_lowering=False, debug=False)

    XT = nc.declare_dram_parameter("XT", [_D, _B], BF16, isOutput=False)
    W1E = nc.declare_dram_parameter("W1E", [_D, _H], BF16, isOutput=False)
    W2E = nc.declare_dram_parameter("W2E", [_H, _D], BF16, isOutput=False)
    B1E = nc.declare_dram_parameter("B1E", [128, _HT], F32, isOutput=False)
    WGXP = nc.declare_dram_parameter("WGXP", [_D, _E], BF16, isOutput=False)
    GBP = nc.declare_dram_parameter("GBP", [_E, 1], F32, isOutput=False)
    OUTT = nc.declare_dram_parameter("OUTT", [_D, _B], BF16, isOutput=True)
    EROW = nc.declare_dram_parameter("EROW", [1, _B], BF16, isOutput=True)

    with tile.TileContext(nc) as tc, ExitStack() as ctx:
        const = ctx.enter_context(tc.tile_pool(name="const", bufs=1))
        wbp = ctx.enter_context(tc.tile_pool(name="wbp", bufs=4))
        htp = ctx.enter_context(tc.tile_pool(name="htp", bufs=24))
        op = ctx.enter_context(tc.tile_pool(name="op", bufs=4))
        pg = ctx.enter_context(tc.tile_pool(name="pg", bufs=2, space="PSUM"))
        ph = ctx.enter_context(tc.tile_pool(name="ph", bufs=4, space="PSUM"))
        po = ctx.enter_context(tc.tile_pool(name="po", bufs=2, space="PSUM"))

        # ---- inputs: minimal DMA descriptors, spread across sync/scalar/
        # gpsimd issue queues --------------------------------------------
        # scalar queue: gating/bias constants
        wgx_sb = const.tile([128, _DT * _E], BF16)
        nc.scalar.dma_start(
            wgx_sb[:],
            bass.AP(
                tensor=WGXP.ap().tensor,
                offset=0,
                ap=[[_E, 128], [128 * _E, _DT], [1, _E]],
            ),
        )
        gb_sb = const.tile([_E, 1], F32)
        nc.scalar.dma_start(gb_sb[:], GBP.ap())
        b1_sb = const.tile([128, _HT], F32)
        nc.scalar.dma_start(b1_sb[:], B1E.ap())

        # x: one [128, B] tile per d-tile; the first chunk loads as a small
        # descriptor so the scalar queue's tiny constants aren't stuck
        # behind megabyte transfers in the shared hardware DMA queues.
        # d0 on sync (+w1), d1 on gpsimd (+w2).
        xd = []
        for d, eng in ((0, nc.sync), (1, nc.gpsimd)):
            t = const.tile([128, _B], BF16, tag=f"xd_{d}")
            eng.dma_start(
                t[:, 0:_CHUNK], XT.ap()[d * 128 : (d + 1) * 128, 0:_CHUNK]
            )
            xd.append(t)
        xm = {(d, c): xd[d][:, c * _CHUNK : (c + 1) * _CHUNK]
              for d in range(_DT) for c in range(_NCH)}

        w1 = []
        for d in range(_DT):
            w1_t = const.tile([128, _H], BF16, tag=f"w1_{d}")
            nc.sync.dma_start(w1_t[:], W1E.ap()[d * 128 : (d + 1) * 128, :])
            w1.append(w1_t)

        # rest of x
        for d, eng in ((0, nc.sync), (1, nc.gpsimd)):
            eng.dma_start(
                xd[d][:, _CHUNK : _B // 2],
                XT.ap()[d * 128 : (d + 1) * 128, _CHUNK : _B // 2],
            )
            eng.dma_start(
                xd[d][:, _B // 2 : _B],
                XT.ap()[d * 128 : (d + 1) * 128, _B // 2 : _B],
            )

        # W2 [H, D] -> [128, HT*D] via 3D AP: group hh = rows hh*128..+128
        w2_all = const.tile([128, _HT * _D], BF16)
        nc.gpsimd.dma_start(
            w2_all[:],
            bass.AP(
                tensor=W2E.ap().tensor,
                offset=0,
                ap=[[_D, 128], [128 * _D, _HT], [1, _D]],
            ),
        )

        # ---- gating chunk: unnormalized own-expert exp row -> wb broadcast.
        # Softmax normalization and the active mask move to the host-side
        # unshard: out = (sum_e m_e * (E_e . eo_e)) / S shares one
        # denominator S across experts, and each core exports its exp row
        # (EROW), from which the host reconstructs S, w, and the mask.
        e_all = const.tile([_E, _B], BF16)
        wb_tiles = {}

        def gating(c):
            psg = pg.tile([_E, _CHUNK], F32, tag="pg")
            for d in range(_DT):
                nc.tensor.matmul(
                    psg[:], wgx_sb[:, d * _E : (d + 1) * _E], xm[(d, c)],
                    start=(d == 0), stop=(d == _DT - 1),
                )
            cs = slice(c * _CHUNK, (c + 1) * _CHUNK)
            nc.scalar.activation(e_all[:, cs], psg[:], AF.Exp, bias=gb_sb[:])
            if c % 2 == 1:
                ps2 = slice((c - 1) * _CHUNK, (c + 1) * _CHUNK)
                wb = wbp.tile([128, 2 * _CHUNK], BF16, tag="wb")
                nc.gpsimd.partition_broadcast(wb[:], e_all[0:1, ps2], 128)
                wb_tiles[c - 1] = wb[:, 0:_CHUNK]
                wb_tiles[c] = wb[:, _CHUNK : 2 * _CHUNK]

        # ---- main, software-pipelined on PE: mm1(c+1) precedes mm2(c) ------
        ht_by_chunk = {}

        def mm1(c):
            ht_tiles = []
            for hh in range(_HT):
                psh = ph.tile([128, _CHUNK], F32, tag="psh")
                for d in range(_DT):
                    nc.tensor.matmul(
                        psh[:],
                        w1[d][:, hh * 128 : (hh + 1) * 128],
                        xm[(d, c)],
                        start=(d == 0), stop=(d == _DT - 1),
                    )
                ht = htp.tile([128, _CHUNK], BF16, tag="ht")
                nc.scalar.activation(
                    ht[:], psh[:], AF.Tanh, bias=b1_sb[:, hh : hh + 1]
                )
                ht_tiles.append(ht)
            ht_by_chunk[c] = ht_tiles

        obuf = {}

        def mm2(c):
            # outputs batch 2 chunks per descriptor except the final pair:
            # small per-chunk descriptors there keep the post-compute DMA
            # tail short (a trailing 1MB transfer costs ~2.6us before the
            # teardown barrier can pass).
            batch = c < _NCH - 2
            half = c % 2
            ht_tiles = ht_by_chunk.pop(c)
            for d2 in range(_DT):
                pso = po.tile([128, _CHUNK], F32, tag="pso")
                for hh in range(_HT):
                    nc.tensor.matmul(
                        pso[:],
                        w2_all[:, hh * _D + d2 * 128 : hh * _D + (d2 + 1) * 128],
                        ht_tiles[hh][:],
                        start=(hh == 0), stop=(hh == _HT - 1),
                    )
                if batch:
                    if half == 0:
                        osb_t = op.tile([128, 2 * _CHUNK], BF16, tag="osb")
                        obuf[d2] = osb_t
                    osb = obuf[d2]
                    nc.vector.tensor_tensor(
                        osb[:, half * _CHUNK : (half + 1) * _CHUNK],
                        pso[:], wb_tiles[c], ALU.mult,
                    )
                    if half == 1:
                        nc.sync.dma_start(
                            OUTT.ap()[
                                d2 * 128 : (d2 + 1) * 128,
                                (c - 1) * _CHUNK : (c + 1) * _CHUNK,
                            ],
                            osb[:],
                        )
                else:
                    osb_t = op.tile([128, _CHUNK], BF16, tag="osb1")
                    nc.vector.tensor_tensor(
                        osb_t[:], pso[:], wb_tiles[c], ALU.mult
                    )
                    eng = nc.sync if d2 == 0 else nc.gpsimd
                    eng.dma_start(
                        OUTT.ap()[
                            d2 * 128 : (d2 + 1) * 128,
                            c * _CHUNK : (c + 1) * _CHUNK,
                        ],
                        osb_t[:],
                    )

        gating(0)
        mm1(0)
        gating(1)
        mm1(1)
        for c in range(_NCH):
            if c + 2 < _NCH:
                gating(c + 2)
                if c + 2 == _NCH - 1:
                    # EROW only needs the last Exp; issuing it here keeps it
                    # off the gpsimd queue's tail behind the final stores
                    nc.gpsimd.dma_start(EROW.ap()[0:1, :], e_all[0:1, :])
            if c + 2 < _NCH:
                mm1(c + 2)
            mm2(c)

    nc.finalize()
    return nc


def _get_nc():
    if "nc" not in _CACHE:
        _CACHE["nc"] = _build()
    return _CACHE["nc"]


def _make_in_maps(t, x, W1, b1, W2, b2, Wg, bg):
    import ml_dtypes

    bf16 = ml_dtypes.bfloat16
    xT = np.ascontiguousarray(x.T.astype(bf16))
    wgx = np.asarray(Wg[:_D], dtype=np.float32)
    gb = (np.float32(t[0]) * Wg[2 * _D] + bg).astype(np.float32)
    in_maps = []
    for c in range(_NCORES):
        perm = [c] + [e for e in range(_E) if e != c]
        in_maps.append(
            {
                "XT": xT,
                "W1E": np.ascontiguousarray(W1[c].astype(bf16)),
                "W2E": np.ascontiguousarray(W2[c].astype(bf16)),
                "B1E": np.ascontiguousarray(
                    b1[c].reshape(_HT, 128).T, dtype=np.float32
                ),
                "WGXP": np.ascontiguousarray(wgx[:, perm].astype(bf16)),
                "GBP": np.ascontiguousarray(gb[perm].reshape(_E, 1)),
            }
        )
    return in_maps


def _assemble(results, inputs):
    # reconstruct softmax denominator and active mask from the exported
    # per-core exp rows; device partials carry the unnormalized E weight
    E = np.stack(
        [np.asarray(results[c]["EROW"]).astype(np.float64).reshape(_B)
         for c in range(_NCORES)]
    )  # [E, B]
    S = E.sum(axis=0)  # [B]
    w = E / S
    out = np.zeros((_B, _D), dtype=np.float64)
    for c in range(_NCORES):
        if (w[c] > _THRESH).any():
            out += results[c]["OUTT"].astype(np.float64).T
    out /= S[:, None]
    b2 = np.asarray(inputs["b2"])
    if np.any(b2):
        # rank-1 bias term sum_e m_e * w[:,e] b2[e,:] — numpy gating replay
        t, x, Wg, bg = (np.asarray(inputs[k]) for k in ("t", "x", "Wg", "bg"))
        logits = x.astype(np.float64) @ Wg[:_D].astype(np.float64)
        logits += np.float64(t[0]) * Wg[2 * _D].astype(np.float64) + bg
        ex = np.exp(logits - logits.max(axis=1, keepdims=True))
        w = ex / ex.sum(axis=1, keepdims=True)
        active = (w > _THRESH).any(axis=0)
        out += (w * active) @ b2.astype(np.float64)
    return out.astype(np.float32)


def run_on_device(t, x, W1, b1, W2, b2, Wg, bg, trace=False):
    from concourse.bass_utils import run_bass_kernel_spmd

    inputs = dict(t=t, x=x, W1=W1, b1=b1, W2=W2, b2=b2, Wg=Wg, bg=bg)
    in_maps = _make_in_maps(**inputs)
    res = run_bass_kernel_spmd(
        _get_nc(), in_maps, list(range(_NCORES)), trace=trace
    )
    return _assemble(res.results, inputs), res


def kernel(t, x, W1, b1, W2, b2, Wg, bg):
    out, _ = run_on_device(t, x, W1, b1, W2, b2, Wg, bg, trace=False)
    return out


# revision 70
# speedup vs baseline: 1.0031x; 1.0031x over previous
"""Trainium2 Bass kernel for nn_ODEFunc_90159953478502 (MoE routing, inference path).

Math (see reference):
    logits  = x @ Wg[:256] + (t*Wg[512] + bg)      # zeros kill Wg[256:512]
    w       = softmax(logits, axis=-1)             # [B, E]
    eo_e    = tanh(x @ W1[e] + b1[e]) @ W2[e] + b2[e]
    active_e = any_b(w[b,e] > 0.01)
    out     = sum_e active_e * w[:,e,None] * eo_e  # softmax max >= 1/8 > 0.01,
                                                   # so >=1 expert always active

Sharding: expert-parallel. Core e holds the full batch plus only W1[e]/W2[e]
and computes the UNNORMALIZED partial E_e[:,None] * (tanh(x@W1[e]+b1[e]) @
W2[e]) in transposed layout ([D, B]), where E_e = exp(logit_e). Because
out = (sum_e m_e * E_e . eo_e) / S shares one softmax denominator S across
experts, normalization and the 0/1 active mask move to the host-side
unshard: each core exports its exp row (EROW), the host reconstructs
S = sum_e E_e, w = E/S, the mask, and divides once. The b2 rank-1 term
(zero here) is added host-side from a numpy gating pass when nonzero.

Device structure per core (all matmuls bf16, 1 cycle/row on PE; steady-state
matmul cadence measured at 216ns per [128,512]-row stream):
  - x arrives pre-transposed (xT [D, B] bf16) so W1/W2/Wg act as matmul lhsT
    in natural layout (out = lhsT.T @ rhs, contraction on partitions).
  - gating weights are column-permuted per core so the core's own expert is
    partition row 0 of the [E, B] logits: no selector matmuls needed.
  - per 512-chunk: logits^T [8,512] via 2 bf16 matmuls, ACT Exp with fused
    +gbias (no max-subtract: |logits| <= ~4) emitting bf16 into a slice of
    e_all [8, B]; chunk-pair rows are gpsimd partition-broadcast to wb
    [128,1024] bf16 (no DRAM bounce, no S-matmul, no reciprocal).
  - mm2 drains: out^T tile = psum * wb, one DVE multiply per [128,512] tile;
    stores batch 2 chunks per descriptor except the final pair (small
    descriptors keep the post-compute DMA tail short).
  - DMA descriptors drain through shared hardware queues in global issue
    order (~400GB/s aggregate); descriptor count is minimized (3D APs for
    Wg/W2), the first x chunk loads as a small descriptor so the tiny
    constants aren't stuck behind megabyte transfers, and the scalar queue
    issues no DMAs so the in-order ACT chain starts early.
"""

import sys

if "/opt/trn_rl_repo" not in sys.path:
    sys.path.insert(0, "/opt/trn_rl_repo")

import numpy as np

_B, _D, _H, _E = 4096, 256, 1024, 8
_NCORES = 8
_CHUNK = 512
_NCH = _B // _CHUNK
_DT = _D // 128   # 2 d-tiles
_HT = _H // 128   # 8 h-tiles
_THRESH = 0.01

_CACHE = {}


def _build():
    import concourse.bass as bass
    import concourse.tile as tile
    import concourse.mybir as mybir
    from concourse import bacc
    from contextlib import ExitStack

    F32 = mybir.dt.float32
    F32R = mybir.dt.float32r
    BF16 = mybir.dt.bfloat16
    AF = mybir.ActivationFunctionType
    ALU = mybir.AluOpType
    AX = mybir.AxisListType

    nc = bacc.Bacc("TRN2", target_bir_lowering=False, debug=False)

    XT = nc.declare_dram_parameter("XT", [_D, _B], BF16, isOutput=False)
    W1E = nc.declare_dram_parameter("W1E", [_D, _H], BF16, isOutput=False)
    W2E = nc.declare_dram_parameter("W2E", [_H, _D], BF16, isOutput=False)
    B1E = nc.declare_dram_parameter("B1E", [128, _HT], F32, isOutput=False)
    WGXP = nc.declare_dram_parameter("WGXP", [_D, _E], BF16, isOutput=False)
    GBP = nc.declare_dram_parameter("GBP", [_E, 1], F32, isOutput=False)
    OUTT = nc.declare_dram_parameter("OUTT", [_D, _B], BF16, isOutput=True)
    EROW = nc.declare_dram_parameter("EROW", [1, _B], BF16, isOutput=True)

    with tile.TileContext(nc) as tc, ExitStack() as ctx:
        const = ctx.enter_context(tc.tile_pool(name="const", bufs=1))
        wbp = ctx.enter_context(tc.tile_pool(name="wbp", bufs=4))
        htp = ctx.enter_context(tc.tile_pool(name="htp", bufs=24))
        op = ctx.enter_context(tc.tile_pool(name="op", bufs=4))
        pg = ctx.enter_context(tc.tile_pool(name="pg", bufs=2, space="PSUM"))
        ph = ctx.enter_context(tc.tile_pool(name="ph", bufs=4, space="PSUM"))
        po = ctx.enter_context(tc.tile_pool(name="po", bufs=2, space="PSUM"))

        # ---- inputs: minimal DMA descriptors, spread across sync/scalar/
        # gpsimd issue queues --------------------------------------------
        # scalar queue: gating/bias constants
        wgx_sb = const.tile([128, _DT * _E], BF16)
        nc.scalar.dma_start(
            wgx_sb[:],
            bass.AP(
                tensor=WGXP.ap().tensor,
                offset=0,
                ap=[[_E, 128], [128 * _E, _DT], [1, _E]],
            ),
        )
        gb_sb = const.tile([_E, 1], F32)
        nc.scalar.dma_start(gb_sb[:], GBP.ap())
        b1_sb = const.tile([128, _HT], F32)
        nc.scalar.dma_start(b1_sb[:], B1E.ap())

        # x: one [128, B] tile per d-tile; the first chunk loads as a small
        # descriptor so the scalar queue's tiny constants aren't stuck
        # behind megabyte transfers in the shared hardware DMA queues.
        # d0 on sync (+w1), d1 on gpsimd (+w2).
        xd = []
        for d, eng in ((0, nc.sync), (1, nc.gpsimd)):
            t = const.tile([128, _B], BF16, tag=f"xd_{d}")
            eng.dma_start(
                t[:, 0:_CHUNK], XT.ap()[d * 128 : (d + 1) * 128, 0:_CHUNK]
            )
            xd.append(t)
        xm = {(d, c): xd[d][:, c * _CHUNK : (c + 1) * _CHUNK]
              for d in range(_DT) for c in range(_NCH)}

        w1 = []
        for d in range(_DT):
            w1_t = const.tile([128, _H], BF16, tag=f"w1_{d}")
            nc.sync.dma_start(w1_t[:], W1E.ap()[d * 128 : (d + 1) * 128, :])
            w1.append(w1_t)

        # rest of x
        for d, eng in ((0, nc.sync), (1, nc.gpsimd)):
            eng.dma_start(
                xd[d][:, _CHUNK : _B // 2],
                XT.ap()[d * 128 : (d + 1) * 128, _CHUNK : _B // 2],
            )
            eng.dma_start(
                xd[d][:, _B // 2 : _B],
                XT.ap()[d * 128 : (d + 1) * 128, _B // 2 : _B],
            )

        # W2 [H, D] -> [128, HT*D] via 3D AP: group hh = rows hh*128..+128
        w2_all = const.tile([128, _HT * _D], BF16)
        nc.gpsimd.dma_start(
            w2_all[:],
            bass.AP(
                tensor=W2E.ap().tensor,
                offset=0,
                ap=[[_D, 128], [128 * _D, _HT], [1, _D]],
            ),
        )

        # ---- gating chunk: unnormalized own-expert exp row -> wb broadcast.
        # Softmax normalization and the active mask move to the host-side
        # unshard: out = (sum_e m_e * (E_e . eo_e)) / S shares one
        # denominator S across experts, and each core exports its exp row
        # (EROW), from which the host reconstructs S, w, and the mask.
        e_all = const.tile([_E, _B], BF16)
        wb_tiles = {}

        def gating(c):
            psg = pg.tile([_E, _CHUNK], F32, tag="pg")
            for d in range(_DT):
                nc.tensor.matmul(
                    psg[:], wgx_sb[:, d * _E : (d + 1) * _E], xm[(d, c)],
                    start=(d == 0), stop=(d == _DT - 1),
                )
            cs = slice(c * _CHUNK, (c + 1) * _CHUNK)
            nc.scalar.activation(e_all[:, cs], psg[:], AF.Exp, bias=gb_sb[:])
            if c % 2 == 1:
                ps2 = slice((c - 1) * _CHUNK, (c + 1) * _CHUNK)
                wb = wbp.tile([128, 2 * _CHUNK], BF16, tag="wb")
                nc.gpsimd.partition_broadcast(wb[:], e_all[0:1, ps2], 128)
                wb_tiles[c - 1] = wb[:, 0:_CHUNK]
                wb_tiles[c] = wb[:, _CHUNK : 2 * _CHUNK]

        # ---- main, software-pipelined on PE: mm1(c+1) precedes mm2(c) ------
        ht_by_chunk = {}

        def mm1(c):
            ht_tiles = []
            for hh in range(_HT):
                psh = ph.tile([128, _CHUNK], F32, tag="psh")
                for d in range(_DT):
                    nc.tensor.matmul(
                        psh[:],
                        w1[d][:, hh * 128 : (hh + 1) * 128],
                        xm[(d, c)],
                        start=(d == 0), stop=(d == _DT - 1),
                    )
                ht = htp.tile([128, _CHUNK], BF16, tag="ht")
                nc.scalar.activation(
                    ht[:], psh[:], AF.Tanh, bias=b1_sb[:, hh : hh + 1]
                )
                ht_tiles.append(ht)
            ht_by_chunk[c] = ht_tiles

        obuf = {}

        def mm2(c):
            # outputs batch 2 chunks per descriptor except the final pair:
            # small per-chunk descriptors there keep the post-compute DMA
            # tail short (a trailing 1MB transfer costs ~2.6us before the
            # teardown barrier can pass).
            batch = c < _NCH - 2
            half = c % 2
            ht_tiles = ht_by_chunk.pop(c)
            for d2 in range(_DT):
                pso = po.tile([128, _CHUNK], F32, tag="pso")
                for hh in range(_HT):
                    nc.tensor.matmul(
                        pso[:],
                        w2_all[:, hh * _D + d2 * 128 : hh * _D + (d2 + 1) * 128],
                        ht_tiles[hh][:],
                        start=(hh == 0), stop=(hh == _HT - 1),
                    )
                if batch:
                    if half == 0:
                        osb_t = op.tile([128, 2 * _CHUNK], BF16, tag="osb")
                        obuf[d2] = osb_t
                    osb = obuf[d2]
                    nc.vector.tensor_tensor(
                        osb[:, half * _CHUNK : (half + 1) * _CHUNK],
                        pso[:], wb_tiles[c], ALU.mult,
                    )
                    if half == 1:
                        nc.sync.dma_start(
                            OUTT.ap()[
                                d2 * 128 : (d2 + 1) * 128,
                                (c - 1) * _CHUNK : (c + 1) * _CHUNK,
                            ],
                            osb[:],
                        )
                else:
                    osb_t = op.tile([128, _CHUNK], BF16, tag="osb1")
                    nc.vector.tensor_tensor(
                        osb_t[:], pso[:], wb_tiles[c], ALU.mult
                    )
                    eng = nc.sync if d2 == 0 else nc.gpsimd
                    eng.dma_start(
                        OUTT.ap()[
                            d2 * 128 : (d2 + 1) * 128,
                            c * _CHUNK : (c + 1) * _CHUNK,
                        ],
                        osb_t[:],
                    )

        gating(0)
        mm1(0)
        gating(1)
        mm1(1)
        for c in range(_NCH):
            if c + 2 < _NCH:
                gating(c + 2)
                if c + 2 == _NCH - 1:
                    # EROW only needs the last Exp; issuing it here keeps it
                    # off the gpsimd queue's tail behind the final stores
                    nc.gpsimd.dma_start(EROW.ap()[0:1, :], e_all[0:1, :])
            if c + 2 < _NCH:
                mm1(c + 2)
            mm2(c)

    nc.finalize()
    return nc


def _get_nc():
    if "nc" not in _CACHE:
        _CACHE["nc"] = _build()
    return _CACHE["nc"]


def _make_in_maps(t, x, W1, b1, W2, b2, Wg, bg):
    import ml_dtypes

    bf16 = ml_dtypes.bfloat16
    xT = np.ascontiguousarray(x.T.astype(bf16))
    wgx = np.asarray(Wg[:_D], dtype=np.float32)
    gb = (np.float32(t[0]) * Wg[2 * _D] + bg).astype(np.float32)
    in_maps = []
    for c in range(_NCORES):
        perm = [c] + [e for e in range(_E) if e != c]
        in_maps.append(
            {
                "XT": xT,
                "W1E": np.ascontiguousarray(W1[c].astype(bf16)),
                "W2E": np.ascontiguousarray(W2[c].astype(bf16)),
                "B1E": np.ascontiguousarray(
                    b1[c].reshape(_HT, 128).T, dtype=np.float32
                ),
                "WGXP": np.ascontiguousarray(wgx[:, perm].astype(bf16)),
                "GBP": np.ascontiguousarray(gb[perm].reshape(_E, 1)),
            }
        )
    return in_maps


def _assemble(results, inputs):
    # reconstruct softmax denominator and active mask from the exported
    # per-core exp rows; device partials carry the unnormalized E weight
    E = np.stack(
        [np.asarray(results[c]["EROW"]).astype(np.float64).reshape(_B)
         for c in range(_NCORES)]
    )  # [E, B]
    S = E.sum(axis=0)  # [B]
    w = E / S
    out = np.zeros((_B, _D), dtype=np.float64)
    for c in range(_NCORES):
        if (w[c] > _THRESH).any():
            out += results[c]["OUTT"].astype(np.float64).T
    out /= S[:, None]
    b2 = np.asarray(inputs["b2"])
    if np.any(b2):
        # rank-1 bias term sum_e m_e * w[:,e] b2[e,:] — numpy gating replay
        t, x, Wg, bg = (np.asarray(inputs[k]) for k in ("t", "x", "Wg", "bg"))
        logits = x.astype(np.float64) @ Wg[:_D].astype(np.float64)
        logits += np.float64(t[0]) * Wg[2 * _D].astype(np.float64) + bg
        ex = np.exp(logits - logits.max(axis=1, keepdims=True))
        w = ex / ex.sum(axis=1, keepdims=True)
        active = (w > _THRESH).any(axis=0)
        out += (w * active) @ b2.astype(np.float64)
    return out.astype(np.float32)


def run_on_device(t, x, W1, b1, W2, b2, Wg, bg, trace=False):
    from concourse.bass_utils import run_bass_kernel_spmd

    inputs = dict(t=t, x=x, W1=W1, b1=b1, W2=W2, b2=b2, Wg=Wg, bg=bg)
    in_maps = _make_in_maps(**inputs)
    res = run_bass_kernel_spmd(
        _get_nc(), in_maps, list(range(_NCORES)), trace=trace
    )
    return _assemble(res.results, inputs), res


def kernel(t, x, W1, b1, W2, b2, Wg, bg):
    out, _ = run_on_device(t, x, W1, b1, W2, b2, Wg, bg, trace=False)
    return out
